# revision 36
# baseline (speedup 1.0000x reference)
"""Trainium2 Bass kernel for BottleneckAttention (patch attention).

q patches [160, 5120] from z1_hat (non-overlapping 10x4 unfold),
kv patches [5551, 5120] from z2 (overlapping unfold, Hk=91 x Wk=61),
scores = q @ kv.T / 5120, softmax over kv patches, out = attn @ kv,
folded back to [1, 128, 100, 64].

Sharding: 12 kv h-rows (768 flat positions) per core; every core computes
all 160 q columns; host combines with an all-gather softmax using the
centered form f = e - 1 (the exact colsum term is added in fp64 on host).

Per-core kernel (v9): every matmul uses the full 128-wide PE array; all
matmul operands are fp8e4 (~1.6e-3 relative error vs the 2e-2 budget).
  phase 1 computes scores TRANSPOSED [pos, q] via a Karatsuba/Winograd
    F(2,2) scheme over the 4 w-taps: positions are paired, taps split
    into two 2-tap pairs, each computed with 3 multiplies
      m1 = d1*(g0+g1), m2 = (d0-d1)*g0, m3 = (d2-d1)*(-g1 negated in q)
      s_even = m1+m2, s_odd = m1+m3
    so 240 tap-matmuls become 180. The stride-2 sampling lives in three
    host-prepared de-interleaved slabs (z at odd x, zdiff at even/odd x),
    keeping weight loads contiguous. Per pair-block the three m-chains
    accumulate into three 160-col slices of ONE psum bank; VectorE forms
    the even/odd sums, ScalarE applies exp, VectorE applies
    f = (e-1)*mask (mask zeroes invalid w>=61 / h>=91 positions).
  phase 2 computes out TRANSPOSED [(c,i,j), q]: per-tap 6-chunk chains
    consuming the parity-split f ordering against parity-split,
    partition-phase-shifted copies of z2T as the stationary operand.
    PSUM->SBUF copies alternate ScalarE/VectorE; fp16 output.
  denominator = ones-vector matmul, slotted after tile 34.
Input DMAs ride the Activation engine HW-DGE queue in consumption order;
outputs ride the SP queue plus the last two chunks on the Activation
queue so the tail DMAs overlap.
"""

import sys

sys.path.insert(0, "/opt/trn_rl_repo")

import numpy as np
import ml_dtypes

import concourse.bass as bass
import concourse.mybir as mybir

DT = mybir.dt
AF = mybir.ActivationFunctionType
ALU = mybir.AluOpType

# problem geometry (hardcoded from the reference module)
KC, KH, KW = 128, 10, 4
H, W = 100, 64
NH, NW = H // KH, W // KW          # 10, 16
PQ = NH * NW                       # 160 q patches
D = KC * KH * KW                   # 5120
HK, WK = H - KH + 1, W - KW + 1    # 91, 61
NCORES = 8
HPC = 12                           # kv h-rows per core
NPOS = 24 * W                      # 1536 slab positions per core
NOWN = HPC * W                     # 768 owned positions per core
NB = NOWN // 128                   # 6 f chunks of 128 rows
NPB = 3                            # pair-blocks (128 position-pairs each)
NSTEP = 20                         # (i, jp) sweep steps
NIJ = KH * KW                      # 40 (i,j) output taps
NU = NPOS // 2                     # 768 de-interleaved slab length
NK2 = 5                            # ztp chunks per phase copy
SCALE = 1.0 / D

F8 = ml_dtypes.float8_e4m3

_CACHE = {}

# output DMA chunk boundaries (tiles)
OCH = [(0, 8), (8, 16), (16, 24), (24, 32)]


def _build_nc():
    nc = bass.Bass()
    zw_d = nc.declare_dram_parameter("zw", [KC, 3, NU], DT.float8e4, isOutput=False)
    q_d = nc.declare_dram_parameter(
        "qT3", [KC, NSTEP, 3, PQ], DT.float8e4, isOutput=False
    )
    zt_d = nc.declare_dram_parameter(
        "ztp", [128, 2, 12, NK2, KC], DT.float8e4, isOutput=False
    )
    mk_d = nc.declare_dram_parameter("msk", [128, 8], DT.float32, isOutput=False)
    out_d = nc.declare_dram_parameter("out", [KC, NIJ, PQ], DT.float16, isOutput=True)
    den_d = nc.declare_dram_parameter("den", [1, PQ], DT.float32, isOutput=True)

    from contextlib import ExitStack

    ctx = ExitStack()
    with ctx:
        zw_sb = ctx.enter_context(nc.sbuf_tensor([KC, 3, NU], DT.float8e4))
        q_sb = ctx.enter_context(nc.sbuf_tensor([KC, NSTEP, 3, PQ], DT.float8e4))
        zt_sb = ctx.enter_context(nc.sbuf_tensor([128, 2, 12, NK2, KC], DT.float8e4))
        mk_sb = ctx.enter_context(nc.sbuf_tensor([128, 8], DT.float32))
        se_sb = ctx.enter_context(nc.sbuf_tensor([128, NB, 192], DT.float32))
        m1_sb = ctx.enter_context(nc.sbuf_tensor([128, NPB, 192], DT.float32))
        e_sb = ctx.enter_context(nc.sbuf_tensor([128, NB, 192], DT.float32))
        f_sb = ctx.enter_context(nc.sbuf_tensor([128, NB, PQ], DT.float8e4))
        o_sb = ctx.enter_context(nc.sbuf_tensor([128, NIJ, PQ], DT.float16))
        den_sb = ctx.enter_context(nc.sbuf_tensor([1, 192], DT.float32))
        ones_sb = ctx.enter_context(nc.sbuf_tensor([128, 1], DT.float8e4))
        wz = ctx.enter_context(nc.sbuf_tensor([128, 128], DT.float8e4))

        ps = [
            ctx.enter_context(nc.psum_tensor(f"ps{i}", [128, 512], DT.float32))
            for i in range(8)
        ]

        s_wz = ctx.enter_context(nc.semaphore("s_wz"))
        s_i0 = ctx.enter_context(nc.semaphore("s_i0"))
        s_q00 = ctx.enter_context(nc.semaphore("s_q00"))
        s_i01 = ctx.enter_context(nc.semaphore("s_i01"))
        s_i1 = ctx.enter_context(nc.semaphore("s_i1"))
        s_i2 = ctx.enter_context(nc.semaphore("s_i2"))
        s_im = ctx.enter_context(nc.semaphore("s_im"))
        s_iz = ctx.enter_context(nc.semaphore("s_iz"))
        s_p = ctx.enter_context(nc.semaphore("s_p"))
        s_add = ctx.enter_context(nc.semaphore("s_add"))
        s_m1 = ctx.enter_context(nc.semaphore("s_m1"))
        s_cpa = ctx.enter_context(nc.semaphore("s_cpa"))
        s_cpv = ctx.enter_context(nc.semaphore("s_cpv"))
        s_f = ctx.enter_context(nc.semaphore("s_f"))
        s_o = ctx.enter_context(nc.semaphore("s_o"))

        # s_p: sweep1 chain stops 1..6, sweep2 stops 7..9,
        #      tiles 0..34 -> 10..44, den -> 45, tiles 35..39 -> 46..50
        # s_add: pair-block sums, c_idx order, 1..6
        # s_cpa: exps 1..6, even-tile copies 7..24 (g<=34), den 25, 26..27
        # s_cpv: odd-tile copy of tile 2m+1 -> m+1 (1..20)
        def sp_tile(g):
            return 10 + g if g < 35 else 11 + g

        with nc.Block() as block:

            @block.sync
            def _(sync):
                # zw rides this (otherwise idle) queue in parallel with the
                # q chunks on the Activation queue; then outputs
                sync.dma_start(zw_sb[:], zw_d[:]).then_inc(s_i0, 16)
                sync.dma_start(q_sb[:, 0:3], q_d[:, 0:3]).then_inc(s_q00, 16)
                for a, b in OCH:
                    sync.wait_ge(s_cpa, 6 + (b + 1) // 2)
                    sync.wait_ge(s_cpv, b // 2)
                    sl = slice(a, b)
                    sync.dma_start(out_d[:, sl, :], o_sb[:, sl, :]).then_inc(
                        s_o, 16
                    )
                sync.wait_ge(s_cpa, 25)
                sync.dma_start(den_d[:, :], den_sb[0:1, 0:PQ]).then_inc(s_o, 16)
                sync.wait_ge(s_cpa, 26)  # even copies through 36
                sync.wait_ge(s_cpv, 19)  # odd copies through 37
                sync.dma_start(out_d[:, 32:38, :], o_sb[:, 32:38, :]).then_inc(
                    s_o, 16
                )
                sync.wait_ge(s_o, 112)

            @block.tensor
            def _(pe):
                # HAM warmup on the zeroed wz tile while input DMAs land
                pe.wait_ge(s_wz, 1)
                for w_ in range(26):
                    nc.tensor.matmul(
                        ps[7][0:128, 0:128],
                        wz[:, 0:128],
                        wz[:, 0:128],
                        start=(w_ == 0),
                        stop=(w_ == 25),
                    )
                pe.wait_ge(s_i0, 16)  # zw landed; keep PE hot on it
                for w_ in range(6):
                    nc.tensor.matmul(
                        ps[7][0:128, 0:256],
                        zw_sb[:, 0, 0:128],
                        zw_sb[:, 0, 0:256],
                        start=(w_ == 0),
                        stop=(w_ == 5),
                    )
                pe.wait_ge(s_q00, 16)
                # phase 1 sweep 1: pair-blocks 0,1 / sweep 2: pair-block 2.
                # m-chain for (blk, comp) accumulates in ps[blk] cols
                # [160c : 160c+160]; weights = de-interleaved slab windows.
                BANKS = {0: (0, 1, 2), 1: (3, 4, 5), 2: (6, 7, 0)}

                def sweep(blks, first):
                    for s in range(NSTEP):
                        if first:
                            if s == 3:
                                pe.wait_ge(s_i01, 16)
                            elif s == 9:
                                pe.wait_ge(s_i1, 16)
                            elif s == 15:
                                pe.wait_ge(s_i2, 16)
                        elif s == 8:
                            pe.wait_ge(s_add, 2)  # bank 0 drained (B0 sums)
                        i_, jp = s // 2, s % 2
                        ub = 32 * i_ + jp
                        mms = []
                        for blk in blks:
                            base = 128 * blk + ub
                            for c in range(3):
                                if not first and c == 2 and s < 8:
                                    continue  # deferred below
                                mms.append(
                                    nc.tensor.matmul(
                                        ps[BANKS[blk][c]][0:128, 0:PQ],
                                        zw_sb[:, c, base : base + 128],
                                        q_sb[:, s, c, 0:PQ],
                                        start=(s == 0 or (not first and c == 2 and s == 8)),
                                        stop=(s == NSTEP - 1),
                                    )
                                )
                            if not first and 8 <= s < 16:
                                # catch-up: deferred comp-2 MM for step s-8
                                sd = s - 8
                                i_d, jp_d = sd // 2, sd % 2
                                bd = 128 * blk + 32 * i_d + jp_d
                                nc.tensor.matmul(
                                    ps[BANKS[blk][2]][0:128, 0:PQ],
                                    zw_sb[:, 2, bd : bd + 128],
                                    q_sb[:, sd, 2, 0:PQ],
                                    start=False,
                                    stop=False,
                                )
                        if s == NSTEP - 1:
                            for mm in mms:
                                mm.then_inc(s_p, 1)

                sweep((0, 1), True)   # s_p 1..6
                sweep((2,), False)    # s_p 7..9
                # phase 2: out_T[(c,i,j), q] per tap, 6-chunk chains over
                # the parity-split f ordering (c_idx = 2*B + par)
                pe.wait_ge(s_iz, 32)
                PERM = (1, 2, 4, 5, 3, 6, 7, 0)

                def p2_mm(g, ci, start, stop):
                    i_, j_ = g // KW, g % KW
                    B, par = ci // 2, ci % 2
                    pi = (j_ + par) % 2
                    t = (j_ + par - pi) // 2
                    return nc.tensor.matmul(
                        ps[PERM[g % 8]][0:128, 0:PQ],
                        zt_sb[:, pi, 3 * (i_ % 4) + t, B + i_ // 4, :],
                        f_sb[:, ci, 0:PQ],
                        start=start,
                        stop=stop,
                    )

                # tiles 0..6: ci 0..3 now (f0..f3 exist), ci 4..5 after the
                # B2 block's f chunks land — hides the sweep-2 post tail
                pe.wait_ge(s_f, 4)
                for g in range(7):
                    if g == 5:
                        pe.wait_ge(s_m1, 3)   # bank 6 read by B2 m1 copy
                    elif g == 6:
                        pe.wait_ge(s_add, 5)  # bank 7 read by B2 SE sum
                    for ci in range(4):
                        p2_mm(g, ci, ci == 0, False)
                for g in range(7):
                    if g == 0:
                        pe.wait_ge(s_f, 5)
                    p2_mm(g, 4, False, False)
                    if g == 0:
                        pe.wait_ge(s_f, NB)
                    mm = p2_mm(g, 5, False, True)
                    mm.then_inc(s_p, 1)
                for g in range(7, NIJ):
                    i_, j_ = g // KW, g % KW
                    if False:
                        pass
                    elif g >= 8 and g % 4 == 0:
                        pe.wait_ge(s_cpa, g // 2 + 4)
                        pe.wait_ge(s_cpv, g // 2 - 2)
                    if g == 38:
                        pe.wait_ge(s_cpa, 25)  # den copied out of ps[7]
                    for ci in range(NB):
                        mm = p2_mm(g, ci, ci == 0, ci == NB - 1)
                    mm.then_inc(s_p, 1)
                    if g == 34:
                        # denominator: ones.T @ f -> [1, 160] in ps[7]
                        pe.wait_ge(s_wz, 2)    # ones ready
                        pe.wait_ge(s_cpa, 22)  # ps[7] freed (tile 30 copy)
                        for ci in range(NB):
                            mm = nc.tensor.matmul(
                                ps[7][0:1, 0:PQ],
                                ones_sb[0:128, 0:1],
                                f_sb[:, ci, 0:PQ],
                                start=(ci == 0),
                                stop=(ci == NB - 1),
                            )
                        mm.then_inc(s_p, 1)  # s_p = 45

            @block.scalar
            def _(act):
                # input DMAs on the Activation HW-DGE queue, consumption order
                act.dma_start(q_sb[:, 3:9], q_d[:, 3:9]).then_inc(s_i01, 16)
                act.dma_start(q_sb[:, 9:15], q_d[:, 9:15]).then_inc(s_i1, 16)
                act.dma_start(q_sb[:, 15:20], q_d[:, 15:20]).then_inc(s_i2, 16)
                act.dma_start(mk_sb[:], mk_d[:]).then_inc(s_im, 16)
                for h in range(2):
                    sl = slice(6 * h, 6 * (h + 1))
                    act.dma_start(zt_sb[:, :, sl], zt_d[:, :, sl]).then_inc(s_iz, 16)
                for ci in range(NB):
                    act.wait_ge(s_add, ci + 1)
                    nc.scalar.activation(
                        e_sb[:, ci, 0:PQ], se_sb[:, ci, 0:PQ], AF.Exp, scale=SCALE
                    ).then_inc(s_cpa, 1)  # 1..6
                for g in range(0, NIJ, 2):
                    act.wait_ge(s_p, sp_tile(g))
                    nc.scalar.activation(
                        o_sb[:, g, :], ps[(1, 2, 4, 5, 3, 6, 7, 0)[g % 8]][0:128, 0:PQ], AF.Copy
                    ).then_inc(s_cpa, 1)  # evens<=34: 7..24, 36+: 26..27
                    if g == 34:
                        act.wait_ge(s_p, 45)
                        nc.scalar.activation(
                            den_sb[0:1, 0:PQ], ps[7][0:1, 0:PQ], AF.Copy
                        ).then_inc(s_cpa, 1)  # 25
                    if g == 38:
                        act.wait_ge(s_cpa, 27)  # own copies (sim dep credit)
                        act.wait_ge(s_cpv, 20)  # tile 39 copied
                        act.dma_start(
                            out_d[:, 38:40, :], o_sb[:, 38:40, :]
                        ).then_inc(s_o, 16)

            @block.vector
            def _(dve):
                nc.vector.memset(wz[:], 0.0).then_inc(s_wz, 1)
                nc.vector.memset(ones_sb[:], 1.0).then_inc(s_wz, 1)
                dve.wait_ge(s_im, 16)  # mask resident

                BANKS = {0: (0, 1, 2), 1: (3, 4, 5), 2: (6, 7, 0)}

                def psum_sum(blk, par, ci):
                    # s_even = m1+m2; s_odd = m1+m3 (m1 staged via SBUF:
                    # the DVE may read at most one PSUM operand)
                    b1, b2, b3 = BANKS[blk]
                    if par == 0:
                        nc.vector.tensor_copy(
                            m1_sb[:, blk, 0:PQ], ps[b1][0:128, 0:PQ]
                        ).then_inc(s_m1, 1)
                    dve.wait_ge(s_m1, blk + 1)
                    other = b2 if par == 0 else b3
                    nc.vector.tensor_tensor(
                        se_sb[:, ci, 0:PQ],
                        m1_sb[:, blk, 0:PQ],
                        ps[other][0:128, 0:PQ],
                        ALU.add,
                    ).then_inc(s_add, 1)

                def fop(ci):
                    dve.wait_ge(s_cpa, ci + 1)
                    nc.vector.tensor_scalar(
                        f_sb[:, ci, 0:PQ],
                        e_sb[:, ci, 0:PQ],
                        -1.0,
                        mk_sb[:, ci : ci + 1],
                        ALU.add,
                        ALU.mult,
                    ).then_inc(s_f, 1)

                dve.wait_ge(s_p, 3)
                psum_sum(0, 0, 0)
                psum_sum(0, 1, 1)
                dve.wait_ge(s_p, 6)
                psum_sum(1, 0, 2)
                psum_sum(1, 1, 3)
                fop(0)
                fop(1)
                fop(2)
                fop(3)
                dve.wait_ge(s_p, 9)
                psum_sum(2, 0, 4)
                psum_sum(2, 1, 5)
                fop(4)
                fop(5)
                for g in range(1, NIJ, 2):
                    dve.wait_ge(s_p, sp_tile(g))
                    nc.vector.tensor_copy(
                        o_sb[:, g, :], ps[(1, 2, 4, 5, 3, 6, 7, 0)[g % 8]][0:128, 0:PQ]
                    ).then_inc(s_cpv, 1)

    return nc


def _host_prep(z1_hat, z2):
    z1 = np.asarray(z1_hat, dtype=np.float32)[0]   # [128, 100, 64]
    z2a = np.asarray(z2, dtype=np.float32)[0]

    # q winograd transform: per (i, jp): (g0+g1, g0, -g1) for taps 2jp, 2jp+1
    q = z1.reshape(KC, NH, KH, NW, KW).transpose(1, 3, 0, 2, 4).reshape(PQ, D)
    q4 = q.reshape(PQ, KC, KH, KW).transpose(1, 2, 3, 0)   # [128, 10, 4, 160]
    qw = np.zeros((KC, NSTEP, 3, PQ), dtype=np.float32)
    for i in range(KH):
        for jp in range(2):
            g0, g1 = q4[:, i, 2 * jp], q4[:, i, 2 * jp + 1]
            s = 2 * i + jp
            qw[:, s, 0] = g0 + g1
            qw[:, s, 1] = g0
            qw[:, s, 2] = -g1
    qw = np.ascontiguousarray(qw.astype(F8))

    z_pad = np.zeros((KC, 112, W), dtype=np.float32)
    z_pad[:, :H] = z2a

    in_maps = []
    for core in range(NCORES):
        h0 = HPC * core
        slab = z_pad[:, h0 : h0 + 24, :].reshape(KC, NPOS)  # [128, 1536] f32
        zd = np.zeros((KC, NPOS), dtype=np.float32)
        zd[:, : NPOS - 1] = slab[:, : NPOS - 1] - slab[:, 1:]
        zw = np.zeros((KC, 3, NU), dtype=np.float32)
        zw[:, 0] = slab[:, 1::2]        # d1 (odd x)
        zw[:, 1] = zd[:, 0::2]          # d0-d1 (even x)
        zw[:, 2] = zd[:, 1::2]          # d1-d2 negated -> pairs with -g1
        zw = np.ascontiguousarray(zw.astype(F8))

        # parity-split z2T phase copies
        z2T = slab.T                                      # [1536, 128]
        ztp = np.zeros((128, 2, 12, NK2, KC), dtype=F8)
        for pi in range(2):
            zF = z2T[pi::2]                               # [768, 128]
            for a in range(4):
                for t in range(3):
                    ph = 32 * a + t
                    v = zF[ph : ph + NK2 * 128].reshape(NK2, 128, KC)
                    ztp[:, pi, 3 * a + t] = v.transpose(1, 0, 2).astype(F8)

        # masks in parity-split order: c_idx = 2*B + par, row p ->
        # position x = 2*(128*B + p) + par
        msk = np.zeros((128, 8), dtype=np.float32)
        p = np.arange(128)
        for B in range(NPB):
            for par in range(2):
                x = 2 * (128 * B + p) + par
                real = ((x % W) < WK) & ((h0 + x // W) < HK)
                msk[:, 2 * B + par] = real
        in_maps.append(
            {
                "zw": zw,
                "qT3": qw,
                "ztp": np.ascontiguousarray(ztp),
                "msk": msk,
            }
        )

    # colsum[(c,i,j)] = sum of kv rows over real patches, via integral image
    I = np.zeros((KC, H + 1, W + 1), dtype=np.float64)
    I[:, 1:, 1:] = z2a.astype(np.float64).cumsum(axis=1).cumsum(axis=2)
    colsum = np.zeros((KC, KH, KW), dtype=np.float64)
    for i in range(KH):
        for j in range(KW):
            colsum[:, i, j] = (
                I[:, i + HK, j + WK] - I[:, i, j + WK] - I[:, i + HK, j] + I[:, i, j]
            )
    return in_maps, colsum.reshape(KC, NIJ)


def kernel(z1_hat, z2):
    from concourse.bass_utils import run_bass_kernel_spmd

    in_maps, colsum = _host_prep(z1_hat, z2)
    if "nc" not in _CACHE:
        _CACHE["nc"] = _build_nc()
    nc = _CACHE["nc"]
    res = run_bass_kernel_spmd(nc, in_maps, list(range(NCORES)))
    num = colsum[:, :, None].astype(np.float64).copy()     # [128, 40, 1]
    num = np.broadcast_to(num, (KC, NIJ, PQ)).copy()
    den = np.full((PQ,), float(HK * WK), dtype=np.float64)
    for r in res.results:
        num += r["out"].astype(np.float64)
        den += r["den"].astype(np.float64)[0]
    out = num / den[None, None, :]
    # fold: [c, (i,j), q=(nh,nw)] -> [1, 128, 100, 64]
    arr = out.reshape(KC, KH, KW, NH, NW).transpose(0, 3, 1, 4, 2)
    return np.ascontiguousarray(arr.reshape(1, KC, H, W).astype(np.float32))


# revision 37
# speedup vs baseline: 1.0012x; 1.0012x over previous
"""Trainium2 Bass kernel for BottleneckAttention (patch attention).

q patches [160, 5120] from z1_hat (non-overlapping 10x4 unfold),
kv patches [5551, 5120] from z2 (overlapping unfold, Hk=91 x Wk=61),
scores = q @ kv.T / 5120, softmax over kv patches, out = attn @ kv,
folded back to [1, 128, 100, 64].

Sharding: 12 kv h-rows (768 flat positions) per core; every core computes
all 160 q columns; host combines with an all-gather softmax using the
centered form f = e - 1 (the exact colsum term is added in fp64 on host).

Per-core kernel (v9): every matmul uses the full 128-wide PE array; all
matmul operands are fp8e4 (~1.6e-3 relative error vs the 2e-2 budget).
  phase 1 computes scores TRANSPOSED [pos, q] via a Karatsuba/Winograd
    F(2,2) scheme over the 4 w-taps: positions are paired, taps split
    into two 2-tap pairs, each computed with 3 multiplies
      m1 = d1*(g0+g1), m2 = (d0-d1)*g0, m3 = (d2-d1)*(-g1 negated in q)
      s_even = m1+m2, s_odd = m1+m3
    so 240 tap-matmuls become 180. The stride-2 sampling lives in three
    host-prepared de-interleaved slabs (z at odd x, zdiff at even/odd x),
    keeping weight loads contiguous. Per pair-block the three m-chains
    accumulate into three 160-col slices of ONE psum bank; VectorE forms
    the even/odd sums, ScalarE applies exp, VectorE applies
    f = (e-1)*mask (mask zeroes invalid w>=61 / h>=91 positions).
  phase 2 computes out TRANSPOSED [(c,i,j), q]: per-tap 6-chunk chains
    consuming the parity-split f ordering against parity-split,
    partition-phase-shifted copies of z2T as the stationary operand.
    PSUM->SBUF copies alternate ScalarE/VectorE; fp16 output.
  denominator = ones-vector matmul, slotted after tile 34.
Input DMAs ride the Activation engine HW-DGE queue in consumption order;
outputs ride the SP queue plus the last two chunks on the Activation
queue so the tail DMAs overlap.
"""

import sys

sys.path.insert(0, "/opt/trn_rl_repo")

import numpy as np
import ml_dtypes

import concourse.bass as bass
import concourse.mybir as mybir

DT = mybir.dt
AF = mybir.ActivationFunctionType
ALU = mybir.AluOpType

# problem geometry (hardcoded from the reference module)
KC, KH, KW = 128, 10, 4
H, W = 100, 64
NH, NW = H // KH, W // KW          # 10, 16
PQ = NH * NW                       # 160 q patches
D = KC * KH * KW                   # 5120
HK, WK = H - KH + 1, W - KW + 1    # 91, 61
NCORES = 8
HPC = 12                           # kv h-rows per core
NPOS = 24 * W                      # 1536 slab positions per core
NOWN = HPC * W                     # 768 owned positions per core
NB = NOWN // 128                   # 6 f chunks of 128 rows
NPB = 3                            # pair-blocks (128 position-pairs each)
NSTEP = 20                         # (i, jp) sweep steps
NIJ = KH * KW                      # 40 (i,j) output taps
NU = NPOS // 2                     # 768 de-interleaved slab length
NK2 = 5                            # ztp chunks per phase copy
SCALE = 1.0 / D

F8 = ml_dtypes.float8_e4m3

_CACHE = {}

# output DMA chunk boundaries (tiles)
OCH = [(0, 8), (8, 16), (16, 24), (24, 32)]


def _build_nc():
    nc = bass.Bass()
    zw_d = nc.declare_dram_parameter("zw", [KC, 3, NU], DT.float8e4, isOutput=False)
    q_d = nc.declare_dram_parameter(
        "qT3", [KC, NSTEP, 3, PQ], DT.float8e4, isOutput=False
    )
    zt_d = nc.declare_dram_parameter(
        "ztp", [128, 2, 12, NK2, KC], DT.float8e4, isOutput=False
    )
    mk_d = nc.declare_dram_parameter("msk", [128, 8], DT.float32, isOutput=False)
    out_d = nc.declare_dram_parameter("out", [KC, NIJ, PQ], DT.float16, isOutput=True)
    den_d = nc.declare_dram_parameter("den", [1, PQ], DT.float32, isOutput=True)

    from contextlib import ExitStack

    ctx = ExitStack()
    with ctx:
        zw_sb = ctx.enter_context(nc.sbuf_tensor([KC, 3, NU], DT.float8e4))
        q_sb = ctx.enter_context(nc.sbuf_tensor([KC, NSTEP, 3, PQ], DT.float8e4))
        zt_sb = ctx.enter_context(nc.sbuf_tensor([128, 2, 12, NK2, KC], DT.float8e4))
        mk_sb = ctx.enter_context(nc.sbuf_tensor([128, 8], DT.float32))
        se_sb = ctx.enter_context(nc.sbuf_tensor([128, NB, 192], DT.float32))
        m1_sb = ctx.enter_context(nc.sbuf_tensor([128, NPB, 192], DT.float32))
        e_sb = ctx.enter_context(nc.sbuf_tensor([128, NB, 192], DT.float32))
        f_sb = ctx.enter_context(nc.sbuf_tensor([128, NB, PQ], DT.float8e4))
        o_sb = ctx.enter_context(nc.sbuf_tensor([128, NIJ, PQ], DT.float16))
        den_sb = ctx.enter_context(nc.sbuf_tensor([1, 192], DT.float32))
        ones_sb = ctx.enter_context(nc.sbuf_tensor([128, 1], DT.float8e4))
        wz = ctx.enter_context(nc.sbuf_tensor([128, 128], DT.float8e4))

        ps = [
            ctx.enter_context(nc.psum_tensor(f"ps{i}", [128, 512], DT.float32))
            for i in range(8)
        ]

        s_wz = ctx.enter_context(nc.semaphore("s_wz"))
        s_i0 = ctx.enter_context(nc.semaphore("s_i0"))
        s_q00 = ctx.enter_context(nc.semaphore("s_q00"))
        s_i01 = ctx.enter_context(nc.semaphore("s_i01"))
        s_i1 = ctx.enter_context(nc.semaphore("s_i1"))
        s_i2 = ctx.enter_context(nc.semaphore("s_i2"))
        s_im = ctx.enter_context(nc.semaphore("s_im"))
        s_iz = ctx.enter_context(nc.semaphore("s_iz"))
        s_p = ctx.enter_context(nc.semaphore("s_p"))
        s_add = ctx.enter_context(nc.semaphore("s_add"))
        s_m1 = ctx.enter_context(nc.semaphore("s_m1"))
        s_cpa = ctx.enter_context(nc.semaphore("s_cpa"))
        s_cpv = ctx.enter_context(nc.semaphore("s_cpv"))
        s_f = ctx.enter_context(nc.semaphore("s_f"))
        s_o = ctx.enter_context(nc.semaphore("s_o"))

        # s_p: sweep1 chain stops 1..6, sweep2 stops 7..9,
        #      tiles 0..34 -> 10..44, den -> 45, tiles 35..39 -> 46..50
        # s_add: pair-block sums, c_idx order, 1..6
        # s_cpa: exps 1..6, even-tile copies 7..24 (g<=34), den 25, 26..27
        # s_cpv: odd-tile copy of tile 2m+1 -> m+1 (1..20)
        def sp_tile(g):
            return 10 + g if g < 35 else 11 + g

        with nc.Block() as block:

            @block.sync
            def _(sync):
                # zw rides this (otherwise idle) queue in parallel with the
                # q chunks on the Activation queue; then outputs
                sync.dma_start(zw_sb[:], zw_d[:]).then_inc(s_i0, 16)
                for a, b in OCH:
                    sync.wait_ge(s_cpa, 6 + (b + 1) // 2)
                    sync.wait_ge(s_cpv, b // 2)
                    sl = slice(a, b)
                    sync.dma_start(out_d[:, sl, :], o_sb[:, sl, :]).then_inc(
                        s_o, 16
                    )
                sync.wait_ge(s_cpa, 25)
                sync.dma_start(den_d[:, :], den_sb[0:1, 0:PQ]).then_inc(s_o, 16)
                sync.wait_ge(s_cpa, 26)  # even copies through 36
                sync.wait_ge(s_cpv, 19)  # odd copies through 37
                sync.dma_start(out_d[:, 32:38, :], o_sb[:, 32:38, :]).then_inc(
                    s_o, 16
                )
                sync.wait_ge(s_o, 112)

            @block.tensor
            def _(pe):
                # HAM warmup on the zeroed wz tile while input DMAs land
                pe.wait_ge(s_wz, 1)
                for w_ in range(26):
                    nc.tensor.matmul(
                        ps[7][0:128, 0:128],
                        wz[:, 0:128],
                        wz[:, 0:128],
                        start=(w_ == 0),
                        stop=(w_ == 25),
                    )
                pe.wait_ge(s_i0, 16)  # zw landed; keep PE hot on it
                for w_ in range(6):
                    nc.tensor.matmul(
                        ps[7][0:128, 0:256],
                        zw_sb[:, 0, 0:128],
                        zw_sb[:, 0, 0:256],
                        start=(w_ == 0),
                        stop=(w_ == 5),
                    )
                pe.wait_ge(s_q00, 16)
                # phase 1 sweep 1: pair-blocks 0,1 / sweep 2: pair-block 2.
                # m-chain for (blk, comp) accumulates in ps[blk] cols
                # [160c : 160c+160]; weights = de-interleaved slab windows.
                BANKS = {0: (0, 1, 2), 1: (3, 4, 5), 2: (6, 7, 0)}

                def sweep(blks, first):
                    for s in range(NSTEP):
                        if first:
                            if s == 3:
                                pe.wait_ge(s_i01, 16)
                            elif s == 9:
                                pe.wait_ge(s_i1, 16)
                            elif s == 15:
                                pe.wait_ge(s_i2, 16)
                        elif s == 8:
                            pe.wait_ge(s_add, 2)  # bank 0 drained (B0 sums)
                        i_, jp = s // 2, s % 2
                        ub = 32 * i_ + jp
                        mms = []
                        for blk in blks:
                            base = 128 * blk + ub
                            for c in range(3):
                                if not first and c == 2 and s < 8:
                                    continue  # deferred below
                                mms.append(
                                    nc.tensor.matmul(
                                        ps[BANKS[blk][c]][0:128, 0:PQ],
                                        zw_sb[:, c, base : base + 128],
                                        q_sb[:, s, c, 0:PQ],
                                        start=(s == 0 or (not first and c == 2 and s == 8)),
                                        stop=(s == NSTEP - 1),
                                    )
                                )
                            if not first and 8 <= s < 16:
                                # catch-up: deferred comp-2 MM for step s-8
                                sd = s - 8
                                i_d, jp_d = sd // 2, sd % 2
                                bd = 128 * blk + 32 * i_d + jp_d
                                nc.tensor.matmul(
                                    ps[BANKS[blk][2]][0:128, 0:PQ],
                                    zw_sb[:, 2, bd : bd + 128],
                                    q_sb[:, sd, 2, 0:PQ],
                                    start=False,
                                    stop=False,
                                )
                        if s == NSTEP - 1:
                            for mm in mms:
                                mm.then_inc(s_p, 1)

                sweep((0, 1), True)   # s_p 1..6
                sweep((2,), False)    # s_p 7..9
                # phase 2: out_T[(c,i,j), q] per tap, 6-chunk chains over
                # the parity-split f ordering (c_idx = 2*B + par)
                pe.wait_ge(s_iz, 32)
                PERM = (1, 2, 4, 5, 3, 6, 7, 0)

                def p2_mm(g, ci, start, stop):
                    i_, j_ = g // KW, g % KW
                    B, par = ci // 2, ci % 2
                    pi = (j_ + par) % 2
                    t = (j_ + par - pi) // 2
                    return nc.tensor.matmul(
                        ps[PERM[g % 8]][0:128, 0:PQ],
                        zt_sb[:, pi, 3 * (i_ % 4) + t, B + i_ // 4, :],
                        f_sb[:, ci, 0:PQ],
                        start=start,
                        stop=stop,
                    )

                # tiles 0..6: ci 0..3 now (f0..f3 exist), ci 4..5 after the
                # B2 block's f chunks land — hides the sweep-2 post tail
                pe.wait_ge(s_f, 4)
                for g in range(7):
                    if g == 5:
                        pe.wait_ge(s_m1, 3)   # bank 6 read by B2 m1 copy
                    elif g == 6:
                        pe.wait_ge(s_add, 5)  # bank 7 read by B2 SE sum
                    for ci in range(4):
                        p2_mm(g, ci, ci == 0, False)
                for g in range(7):
                    if g == 0:
                        pe.wait_ge(s_f, 5)
                    p2_mm(g, 4, False, False)
                    if g == 0:
                        pe.wait_ge(s_f, NB)
                    mm = p2_mm(g, 5, False, True)
                    mm.then_inc(s_p, 1)
                for g in range(7, NIJ):
                    i_, j_ = g // KW, g % KW
                    if False:
                        pass
                    elif g >= 8 and g % 4 == 0:
                        pe.wait_ge(s_cpa, g // 2 + 4)
                        pe.wait_ge(s_cpv, g // 2 - 2)
                    if g == 38:
                        pe.wait_ge(s_cpa, 25)  # den copied out of ps[7]
                    for ci in range(NB):
                        mm = p2_mm(g, ci, ci == 0, ci == NB - 1)
                    mm.then_inc(s_p, 1)
                    if g == 34:
                        # denominator: ones.T @ f -> [1, 160] in ps[7]
                        pe.wait_ge(s_wz, 2)    # ones ready
                        pe.wait_ge(s_cpa, 22)  # ps[7] freed (tile 30 copy)
                        for ci in range(NB):
                            mm = nc.tensor.matmul(
                                ps[7][0:1, 0:PQ],
                                ones_sb[0:128, 0:1],
                                f_sb[:, ci, 0:PQ],
                                start=(ci == 0),
                                stop=(ci == NB - 1),
                            )
                        mm.then_inc(s_p, 1)  # s_p = 45

            @block.scalar
            def _(act):
                # input DMAs on the Activation HW-DGE queue, consumption order
                act.dma_start(q_sb[:, 0:3], q_d[:, 0:3]).then_inc(s_q00, 16)
                act.dma_start(q_sb[:, 3:9], q_d[:, 3:9]).then_inc(s_i01, 16)
                act.dma_start(q_sb[:, 9:15], q_d[:, 9:15]).then_inc(s_i1, 16)
                act.dma_start(q_sb[:, 15:20], q_d[:, 15:20]).then_inc(s_i2, 16)
                act.dma_start(mk_sb[:], mk_d[:]).then_inc(s_im, 16)
                for h in range(2):
                    sl = slice(6 * h, 6 * (h + 1))
                    act.dma_start(zt_sb[:, :, sl], zt_d[:, :, sl]).then_inc(s_iz, 16)
                for ci in range(NB):
                    act.wait_ge(s_add, ci + 1)
                    nc.scalar.activation(
                        e_sb[:, ci, 0:PQ], se_sb[:, ci, 0:PQ], AF.Exp, scale=SCALE
                    ).then_inc(s_cpa, 1)  # 1..6
                for g in range(0, NIJ, 2):
                    act.wait_ge(s_p, sp_tile(g))
                    nc.scalar.activation(
                        o_sb[:, g, :], ps[(1, 2, 4, 5, 3, 6, 7, 0)[g % 8]][0:128, 0:PQ], AF.Copy
                    ).then_inc(s_cpa, 1)  # evens<=34: 7..24, 36+: 26..27
                    if g == 34:
                        act.wait_ge(s_p, 45)
                        nc.scalar.activation(
                            den_sb[0:1, 0:PQ], ps[7][0:1, 0:PQ], AF.Copy
                        ).then_inc(s_cpa, 1)  # 25
                    if g == 38:
                        act.wait_ge(s_cpa, 27)  # own copies (sim dep credit)
                        act.wait_ge(s_cpv, 20)  # tile 39 copied
                        act.dma_start(
                            out_d[:, 38:40, :], o_sb[:, 38:40, :]
                        ).then_inc(s_o, 16)

            @block.vector
            def _(dve):
                nc.vector.memset(wz[:], 0.0).then_inc(s_wz, 1)
                nc.vector.memset(ones_sb[:], 1.0).then_inc(s_wz, 1)
                dve.wait_ge(s_im, 16)  # mask resident

                BANKS = {0: (0, 1, 2), 1: (3, 4, 5), 2: (6, 7, 0)}

                def psum_sum(blk, par, ci):
                    # s_even = m1+m2; s_odd = m1+m3 (m1 staged via SBUF:
                    # the DVE may read at most one PSUM operand)
                    b1, b2, b3 = BANKS[blk]
                    if par == 0:
                        nc.vector.tensor_copy(
                            m1_sb[:, blk, 0:PQ], ps[b1][0:128, 0:PQ]
                        ).then_inc(s_m1, 1)
                    dve.wait_ge(s_m1, blk + 1)
                    other = b2 if par == 0 else b3
                    nc.vector.tensor_tensor(
                        se_sb[:, ci, 0:PQ],
                        m1_sb[:, blk, 0:PQ],
                        ps[other][0:128, 0:PQ],
                        ALU.add,
                    ).then_inc(s_add, 1)

                def fop(ci):
                    dve.wait_ge(s_cpa, ci + 1)
                    nc.vector.tensor_scalar(
                        f_sb[:, ci, 0:PQ],
                        e_sb[:, ci, 0:PQ],
                        -1.0,
                        mk_sb[:, ci : ci + 1],
                        ALU.add,
                        ALU.mult,
                    ).then_inc(s_f, 1)

                dve.wait_ge(s_p, 3)
                psum_sum(0, 0, 0)
                psum_sum(0, 1, 1)
                dve.wait_ge(s_p, 6)
                psum_sum(1, 0, 2)
                psum_sum(1, 1, 3)
                fop(0)
                fop(1)
                fop(2)
                fop(3)
                dve.wait_ge(s_p, 9)
                psum_sum(2, 0, 4)
                psum_sum(2, 1, 5)
                fop(4)
                fop(5)
                for g in range(1, NIJ, 2):
                    dve.wait_ge(s_p, sp_tile(g))
                    nc.vector.tensor_copy(
                        o_sb[:, g, :], ps[(1, 2, 4, 5, 3, 6, 7, 0)[g % 8]][0:128, 0:PQ]
                    ).then_inc(s_cpv, 1)

    return nc


def _host_prep(z1_hat, z2):
    z1 = np.asarray(z1_hat, dtype=np.float32)[0]   # [128, 100, 64]
    z2a = np.asarray(z2, dtype=np.float32)[0]

    # q winograd transform: per (i, jp): (g0+g1, g0, -g1) for taps 2jp, 2jp+1
    q = z1.reshape(KC, NH, KH, NW, KW).transpose(1, 3, 0, 2, 4).reshape(PQ, D)
    q4 = q.reshape(PQ, KC, KH, KW).transpose(1, 2, 3, 0)   # [128, 10, 4, 160]
    qw = np.zeros((KC, NSTEP, 3, PQ), dtype=np.float32)
    for i in range(KH):
        for jp in range(2):
            g0, g1 = q4[:, i, 2 * jp], q4[:, i, 2 * jp + 1]
            s = 2 * i + jp
            qw[:, s, 0] = g0 + g1
            qw[:, s, 1] = g0
            qw[:, s, 2] = -g1
    qw = np.ascontiguousarray(qw.astype(F8))

    z_pad = np.zeros((KC, 112, W), dtype=np.float32)
    z_pad[:, :H] = z2a

    in_maps = []
    for core in range(NCORES):
        h0 = HPC * core
        slab = z_pad[:, h0 : h0 + 24, :].reshape(KC, NPOS)  # [128, 1536] f32
        zd = np.zeros((KC, NPOS), dtype=np.float32)
        zd[:, : NPOS - 1] = slab[:, : NPOS - 1] - slab[:, 1:]
        zw = np.zeros((KC, 3, NU), dtype=np.float32)
        zw[:, 0] = slab[:, 1::2]        # d1 (odd x)
        zw[:, 1] = zd[:, 0::2]          # d0-d1 (even x)
        zw[:, 2] = zd[:, 1::2]          # d1-d2 negated -> pairs with -g1
        zw = np.ascontiguousarray(zw.astype(F8))

        # parity-split z2T phase copies
        z2T = slab.T                                      # [1536, 128]
        ztp = np.zeros((128, 2, 12, NK2, KC), dtype=F8)
        for pi in range(2):
            zF = z2T[pi::2]                               # [768, 128]
            for a in range(4):
                for t in range(3):
                    ph = 32 * a + t
                    v = zF[ph : ph + NK2 * 128].reshape(NK2, 128, KC)
                    ztp[:, pi, 3 * a + t] = v.transpose(1, 0, 2).astype(F8)

        # masks in parity-split order: c_idx = 2*B + par, row p ->
        # position x = 2*(128*B + p) + par
        msk = np.zeros((128, 8), dtype=np.float32)
        p = np.arange(128)
        for B in range(NPB):
            for par in range(2):
                x = 2 * (128 * B + p) + par
                real = ((x % W) < WK) & ((h0 + x // W) < HK)
                msk[:, 2 * B + par] = real
        in_maps.append(
            {
                "zw": zw,
                "qT3": qw,
                "ztp": np.ascontiguousarray(ztp),
                "msk": msk,
            }
        )

    # colsum[(c,i,j)] = sum of kv rows over real patches, via integral image
    I = np.zeros((KC, H + 1, W + 1), dtype=np.float64)
    I[:, 1:, 1:] = z2a.astype(np.float64).cumsum(axis=1).cumsum(axis=2)
    colsum = np.zeros((KC, KH, KW), dtype=np.float64)
    for i in range(KH):
        for j in range(KW):
            colsum[:, i, j] = (
                I[:, i + HK, j + WK] - I[:, i, j + WK] - I[:, i + HK, j] + I[:, i, j]
            )
    return in_maps, colsum.reshape(KC, NIJ)


def kernel(z1_hat, z2):
    from concourse.bass_utils import run_bass_kernel_spmd

    in_maps, colsum = _host_prep(z1_hat, z2)
    if "nc" not in _CACHE:
        _CACHE["nc"] = _build_nc()
    nc = _CACHE["nc"]
    res = run_bass_kernel_spmd(nc, in_maps, list(range(NCORES)))
    num = colsum[:, :, None].astype(np.float64).copy()     # [128, 40, 1]
    num = np.broadcast_to(num, (KC, NIJ, PQ)).copy()
    den = np.full((PQ,), float(HK * WK), dtype=np.float64)
    for r in res.results:
        num += r["out"].astype(np.float64)
        den += r["den"].astype(np.float64)[0]
    out = num / den[None, None, :]
    # fold: [c, (i,j), q=(nh,nw)] -> [1, 128, 100, 64]
    arr = out.reshape(KC, KH, KW, NH, NW).transpose(0, 3, 1, 4, 2)
    return np.ascontiguousarray(arr.reshape(1, KC, H, W).astype(np.float32))
